# revision 25
# baseline (speedup 1.0000x reference)
"""CAM (channel attention) module kernel for Trainium2, 8-core data-parallel.

Reference computation (per sample, C=512, HW=4096):
    v = x.reshape(C, HW)
    E = v @ v.T                                  # (C, C)
    att = softmax(rowmax(E) - E, axis=-1)        # == softmax(-E) stabilized at rowmin
    o = att @ v                                  # (C, HW)
    o = softmax(o, axis=-1)
    out = x + gamma * o
Sharding: data-parallel over batch B=16 -> 2 samples per NeuronCore.

v4: host ships three layouts of x so the device does zero data reshaping:
- xb  (S, C, HW) bf16        natural, residual add only
- vq  (S, 2, 128, 2, HW) fp8 natural, DoubleRow rhs layout for mm2
      vq[s,u,p,ko,n] = x[s, u*256+ko*128+p, n]
- vt  (S, 128, 16, 2, C) fp8 transposed, DoubleRow layout for mm1
      vt[s,p,t,ko,c] = x[s, c, t*256+ko*128+p]
This removes all PE identity-matmul transposes of v (~21 us/core), their
PSUM->SBUF evictions (~17 us DVE), and all SWDGE cast-DMA traffic; every
DMA is a plain large HWDGE transfer.  mm1 is computed full-square (it is
LDWEIGHTS-bound under DoubleRow, so the symmetric-half trick saved no PE
time).  att is written as fp8 directly by the ACT exp, transposed with
cheap fp8 FWL matmuls.  The final out = x + (gamma/Z2)*exp is a single
scalar_tensor_tensor pass; sample 0's finals run on the otherwise idle
GPSIMD so the DVE never queue-blocks sample 1's softmax chain.
"""

import sys

if "/opt/trn_rl_repo" not in sys.path:
    sys.path.insert(0, "/opt/trn_rl_repo")

from contextlib import ExitStack

import numpy as np

P = 128
C = 512
HW = 4096
S = 2  # samples per core
CB = C // P  # 4 channel blocks
NT = HW // (2 * P)  # 16 DoubleRow k-groups (256 contraction each) for mm1
NU = CB // 2  # 2 DoubleRow k-groups for mm2 (channel contraction)
NJ = HW // 1024  # 4 psum chunks (2 banks each) for mm2 output
N_CORES = 8

_NC = None


def _build_nc():
    import concourse.bacc as bacc
    import concourse.mybir as mybir
    import concourse.tile as tile
    from concourse.masks import make_identity

    f32 = mybir.dt.float32
    bf16 = mybir.dt.bfloat16
    fp8 = mybir.dt.float8e4
    AF = mybir.ActivationFunctionType
    ALU = mybir.AluOpType
    AX = mybir.AxisListType
    DR = mybir.MatmulPerfMode.DoubleRow

    nc = bacc.Bacc(
        "TRN2",
        target_bir_lowering=False,
        debug=False,
        num_devices=N_CORES,
    )
    xb = nc.dram_tensor("xb", (S, C, HW), bf16, kind="ExternalInput").ap()
    vq = nc.dram_tensor("vq", (S, NU, P, 2, HW), fp8, kind="ExternalInput").ap()
    vt = nc.dram_tensor("vt", (S, P, NT, 2, C), fp8, kind="ExternalInput").ap()
    gamma = nc.dram_tensor("gamma", (1,), f32, kind="ExternalInput").ap()
    out = nc.dram_tensor("out", (S, C, HW), bf16, kind="ExternalOutput").ap()

    with tile.TileContext(nc) as tc, ExitStack() as ctx:
        const = ctx.enter_context(tc.tile_pool(name="const", bufs=1))
        ident8 = const.tile([P, P], fp8)
        make_identity(nc, ident8)
        gamma_sb = const.tile([P, 1], f32)
        nc.sync.dma_start(out=gamma_sb, in_=gamma.to_broadcast((P, 1)))

        vt_pool = ctx.enter_context(tc.tile_pool(name="vt_pool", bufs=2))
        vq_pool = ctx.enter_context(tc.tile_pool(name="vq_pool", bufs=2 * NU))
        xb_pool = ctx.enter_context(tc.tile_pool(name="xb_pool", bufs=2 * CB))
        att_pool = ctx.enter_context(tc.tile_pool(name="att_pool", bufs=CB + 2))
        attT_pool = ctx.enter_context(tc.tile_pool(name="attT_pool", bufs=2 * CB))
        er_pool = ctx.enter_context(tc.tile_pool(name="er_pool", bufs=4))
        small = ctx.enter_context(tc.tile_pool(name="small", bufs=16))
        # PSUM: psE = single-bank E rows for the interleaved mm1(s1);
        # psB = [P,1024] 2-bank slots (E pairs of mm1(s0), o2 chunks, attT).
        # Static total 2*1 + 3*2 = 8 banks.
        psE = ctx.enter_context(tc.tile_pool(name="psE", bufs=2, space="PSUM"))
        psB = ctx.enter_context(tc.tile_pool(name="psB", bufs=3, space="PSUM"))

        # per-sample state
        vts = [None] * S
        vqs = [[None] * NU for _ in range(S)]
        xbs = [[None] * CB for _ in range(S)]
        att8 = [[None] * CB for _ in range(S)]
        attT = [[None] * CB for _ in range(S)]
        r1s = [[None] * CB for _ in range(S)]

        # Loads alternate between the two HWDGE rings (sync + scalar) —
        # one ring drains ~235 GB/s, two in parallel roughly double the
        # early-load rate.  Only early loads ride the scalar ring so load
        # posts never contend with exp1/exp2 on the ACT queue.
        def vt_load(s, nchunks, engs=(nc.sync,)):
            # contiguous chunk ranges per ring (NOT alternating): mm1
            # consumes t in order, so the early t's must all sit on one
            # ring's in-order stream while the other ring prefetches the
            # tail chunks in parallel.
            t_ = vt_pool.tile([P, NT, 2, C], fp8, tag="vt", name=f"vt_{s}")
            vts[s] = t_
            step = NT // nchunks
            chunks = list(range(0, NT, step))
            half = (len(chunks) + 1) // 2
            for k, c0 in enumerate(chunks):
                eng = engs[0] if k < half or len(engs) == 1 else engs[1]
                eng.dma_start(
                    out=t_[:, c0 : c0 + step], in_=vt[s, :, c0 : c0 + step]
                )

        def vq_load(s, engs=(nc.sync,)):
            for u in range(NU):
                t_ = vq_pool.tile([P, 2, HW], fp8, tag="vq", name=f"vq_{s}_{u}")
                engs[u % len(engs)].dma_start(out=t_, in_=vq[s, u])
                vqs[s][u] = t_

        def xb_load(s, engs=(nc.sync,)):
            for i in range(CB):
                t_ = xb_pool.tile([P, HW], bf16, tag="xb", name=f"xb_{s}_{i}")
                engs[i % len(engs)].dma_start(
                    out=t_, in_=xb[s, i * P : (i + 1) * P, :]
                )
                xbs[s][i] = t_

        def softmax1(s, i, E):
            # att row-block i: exp(rowmin - E) in fp8, r1 = 1/Z1
            m = small.tile([P, 1], f32, tag="sm", name=f"m_{s}_{i}")
            nc.vector.tensor_reduce(m, E, axis=AX.X, op=ALU.min)
            a = att_pool.tile([P, C], fp8, tag="att", name=f"att_{s}_{i}")
            z1 = small.tile([P, 1], f32, tag="sm", name=f"z1_{s}_{i}")
            nc.scalar.activation(a, E, AF.Exp, bias=m, scale=-1.0, accum_out=z1)
            r1 = small.tile([P, 1], f32, tag="sm", name=f"r1_{s}_{i}")
            nc.vector.reciprocal(r1, z1)
            att8[s][i] = a
            r1s[s][i] = r1

        def mm1_mm(s, i, Ei, t):
            nc.tensor.matmul(
                Ei,
                lhsT=vts[s][:, t, :, i * P : (i + 1) * P],
                rhs=vts[s][:, t],
                perf_mode=DR,
                start=(t == 0),
                stop=(t == NT - 1),
            )

        def mm1_s0():
            # E = v v^T full square for sample 0, E row-pairs in psB slots.
            # t-outer over the first six vt DMA chunks so the PE chases the
            # loads with no long idle; the last two chunks go i-outer so the
            # four softmax tails drain staggered instead of serializing.
            E01 = psB.tile([P, 2, C], f32, tag="ps", name="E01_0")
            E23 = psB.tile([P, 2, C], f32, tag="ps", name="E23_0")
            Ei = lambda i: (E01 if i < 2 else E23)[:, i % 2, :]
            for t in range(12):
                for i in range(CB):
                    mm1_mm(0, i, Ei(i), t)
            for i in range(CB):
                for t in range(12, NT):
                    mm1_mm(0, i, Ei(i), t)
                softmax1(0, i, Ei(i))

        def mm1_row(s, i):
            # one E row-block in a single psE bank (pipelined phase)
            E = psE.tile([P, C], f32, tag="psE", name=f"E_{s}_{i}")
            for t in range(NT):
                mm1_mm(s, i, E, t)
            softmax1(s, i, E)

        def attT_row(s, i):
            # attR[s][i][p, jj, m] = att[i*128+m, jj*128+p]: the only slice of
            # att^T that mm2(s,i) needs, so each row's transpose follows its
            # own softmax instead of barriering on all four.  4 fp8 FWL
            # matmul transposes into one psum bank, one small eviction.
            pa = psE.tile([P, 2 * NU, P], f32, tag="psE", name=f"pa_{s}_{i}")
            for jj in range(2 * NU):
                nc.tensor.matmul(
                    pa[:, jj, :],
                    lhsT=att8[s][i][:, jj * P : (jj + 1) * P],
                    rhs=ident8,
                    start=True,
                    stop=True,
                )
            t_ = attT_pool.tile([P, 2 * NU, P], fp8, tag="attT", name=f"attR_{s}_{i}")
            nc.vector.tensor_copy(t_, pa)
            attT[s][i] = t_

        def mm2(s, i):
            # o = att @ v (DoubleRow over channel pairs), softmax over HW with
            # 1/Z1 folded into the exp scale; finals emitted by caller order.
            er = er_pool.tile([P, HW], bf16, tag="er", name=f"er_{s}_{i}")
            z2p = small.tile([P, NJ], f32, tag="z2p", name=f"z2p_{s}_{i}")
            for nj in range(NJ):
                o2 = psB.tile([P, 1024], f32, tag="ps", name=f"o2_{s}_{i}_{nj}")
                for hh in range(2):
                    sl = slice(nj * 1024 + hh * 512, nj * 1024 + (hh + 1) * 512)
                    for u in range(NU):
                        nc.tensor.matmul(
                            o2[:, hh * 512 : (hh + 1) * 512],
                            lhsT=attT[s][i][:, 2 * u : 2 * u + 2, :],
                            rhs=vqs[s][u][:, :, sl],
                            perf_mode=DR,
                            start=(u == 0),
                            stop=(u == NU - 1),
                        )
                nc.scalar.activation(
                    er[:, nj * 1024 : (nj + 1) * 1024],
                    o2,
                    AF.Exp,
                    scale=r1s[s][i],
                    accum_out=z2p[:, nj : nj + 1],
                )
            z2 = small.tile([P, 1], f32, tag="sm", name=f"z2_{s}_{i}")
            nc.vector.reduce_sum(z2, z2p, axis=AX.X)
            r2 = small.tile([P, 1], f32, tag="sm", name=f"r2_{s}_{i}")
            nc.vector.reciprocal(r2, z2)
            gz = small.tile([P, 1], f32, tag="sm", name=f"gz_{s}_{i}")
            nc.vector.tensor_scalar_mul(gz, r2, gamma_sb)
            return er, gz

        def finals(s, i, er, gz, nch=2):
            # out = x + (gamma/Z2)*er: scale er by gz in place (packed
            # tensor_scalar), all-bf16 tensor_tensor add (packed), store the
            # chunk — stores overlap the remaining adds.  (GPSIMD tensor ops
            # measured ~40x slower than DVE, and DVE scalar_tensor_tensor is
            # a ~20x slow path — both were tried and rejected.)  Sample 1
            # drains at [P,1024] granularity to shorten the tail.
            xt = xbs[s][i]
            for h in range(nch):
                sl = slice(h * (HW // nch), (h + 1) * (HW // nch))
                nc.vector.tensor_scalar_mul(er[:, sl], er[:, sl], gz)
                nc.vector.tensor_tensor(
                    out=xt[:, sl], in0=er[:, sl], in1=xt[:, sl], op=ALU.add
                )
                nc.sync.dma_start(out=out[s, i * P : (i + 1) * P, sl], in_=xt[:, sl])

        # ---- load stream (one in-order HWDGE ring, ordered by first use:
        # mm1(s0) ~4us, mm2(s0,0) ~17us, mm1(s1,0) ~21us, finals(s0,0)
        # ~26us, mm2(s1,0) ~50us, finals(s1,0) ~58us) ----
        both = (nc.sync, nc.scalar)
        vt_load(0, nchunks=8, engs=both)
        vq_load(0, engs=both)
        vt_load(1, nchunks=2, engs=both)
        xb_load(0, engs=both)
        vq_load(1)
        xb_load(1)

        # ---- software pipeline across the two samples ----
        # Coarse s0/s1 row interleave (a finer all-rows pipeline was tried
        # and lost ~13us to SBUF/PSUM port contention: every op slowed
        # 20-25% when all engines ran dense).  mm1(s1) rows fill the PE
        # between mm2(s0) rows; mm2(1,0) is pulled ahead of mm1_row(1,3)
        # so the ACT-only tail is 3 exp2 rows instead of 4.  attT_row is a
        # step behind its softmax so the PE never waits on exp1; finals(0,i)
        # come after mm1_row(1,i)'s softmax so the DVE min-reduce is never
        # queued behind them.
        mm1_s0()
        attT_row(0, 0)
        f0 = [None] * CB
        for i in range(3):
            attT_row(0, i + 1)
            f0[i] = mm2(0, i)
            if i >= 1:
                attT_row(1, i - 1)
            mm1_row(1, i)
            finals(0, i, *f0[i])
        f0[3] = mm2(0, 3)
        attT_row(1, 2)
        f10 = mm2(1, 0)
        mm1_row(1, 3)
        finals(0, 3, *f0[3])
        attT_row(1, 3)
        finals(1, 0, *f10, nch=4)
        for i in range(1, CB):
            f1 = mm2(1, i)
            finals(1, i, *f1, nch=4)

    nc.compile()
    return nc


def get_nc():
    global _NC
    if _NC is None:
        _NC = _build_nc()
    return _NC


def _prep_inputs(x: np.ndarray, gamma: np.ndarray):
    """Host-side layout prep: bf16 natural + two fp8 DoubleRow layouts."""
    import ml_dtypes

    B = x.shape[0]
    v = np.asarray(x, dtype=np.float32).reshape(B, C, HW)
    xb = v.astype(ml_dtypes.bfloat16)
    v8 = v.astype(ml_dtypes.float8_e4m3)
    # vq[b, u, p, ko, n] = v8[b, u*256 + ko*128 + p, n]
    vq = np.ascontiguousarray(
        v8.reshape(B, NU, 2, P, HW).transpose(0, 1, 3, 2, 4)
    )
    # vt[b, p, t, ko, c] = v8[b, c, t*256 + ko*128 + p]
    vt = np.ascontiguousarray(
        v8.reshape(B, C, NT, 2, P).transpose(0, 4, 2, 3, 1)
    )
    g = np.ascontiguousarray(np.asarray(gamma, dtype=np.float32)).reshape(1)
    in_maps = [
        {
            "xb": xb[S * c : S * (c + 1)],
            "vq": vq[S * c : S * (c + 1)],
            "vt": vt[S * c : S * (c + 1)],
            "gamma": g,
        }
        for c in range(N_CORES)
    ]
    return in_maps


def kernel(x: np.ndarray, gamma: np.ndarray) -> np.ndarray:
    from concourse.bass_utils import run_bass_kernel_spmd

    B, Cx, H, W = x.shape
    assert (B, Cx, H * W) == (16, C, HW), (B, Cx, H, W)
    nc = get_nc()
    in_maps = _prep_inputs(x, gamma)
    res = run_bass_kernel_spmd(nc, in_maps, core_ids=list(range(N_CORES)))
    out = np.concatenate([res.results[c]["out"] for c in range(N_CORES)], axis=0)
    return out.astype(np.float32).reshape(B, Cx, H, W)
